# revision 1
# baseline (speedup 1.0000x reference)
"""Causal single-head attention (B=4, S=2048, D=1024) on 8 trn2 NeuronCores.

Sharding: core = (batch b, parity h).  Each core owns the 1024 queries of
batch b in 256-row blocks {2t+h : t=0..3} (interleaved for causal load
balance) and computes its own K/V projections for the full sequence.

On-chip dataflow (per core, SPMD-uniform):
  phase 0: QT[e,q]  = Wq^T x_q^T        (bf16 matmuls, fp32 psum)
  phase 1: KT[e,k]  = Wk^T x^T,  V[k,e] = x Wv   (4 key chunks of 512)
  phase 2: per slot t (q-block 2t+h, 256 queries):
       scoresT[k,q] = KT^T QT   over k-tiles 0..4t+3   (keys 0..512(t+1))
       expT = exp(scoresT/32) * mask   (multiplicative 0/1 causal mask on
                                        the last 4 k-tiles of the slot)
       denom[1,q] += ones^T expT       (sum over keys via matmul)
       outT[e,q]  += V^T expT          (PV accumulate in psum)
       out[q,e] = transpose(outT) * (1/denom)   (PE transpose + DVE scale)
"""

import os
import sys
from collections import deque
from contextlib import ExitStack

import numpy as np
import ml_dtypes

import concourse.bass as bass
import concourse.mybir as mybir
import concourse.tile as tile
from concourse import bacc
from concourse import bass_utils
from concourse.masks import make_identity

B, S, D = 4, 2048, 1024
P = 128
QB = 256          # queries per slot
NSLOT = 4         # slots per core
NQ = QB * NSLOT   # queries per core
NCORES = 8
F32 = mybir.dt.float32
BF16 = mybir.dt.bfloat16
SCALE = 1.0 / 32.0  # 1/sqrt(D)


def _build_kernel():
    nc = bacc.Bacc("TRN2", target_bir_lowering=False, debug=False,
                   num_devices=NCORES)

    xt4 = nc.dram_tensor("xt4", [8, P, 8, 256], F32, kind="ExternalInput").ap()
    xq4 = nc.dram_tensor("xq4", [4, P, 8, 256], F32, kind="ExternalInput").ap()
    wq4 = nc.dram_tensor("wq4", [4, P, 8, 256], F32, kind="ExternalInput").ap()
    wk4 = nc.dram_tensor("wk4", [4, P, 8, 256], F32, kind="ExternalInput").ap()
    wv4 = nc.dram_tensor("wv4", [4, P, 8, 256], F32, kind="ExternalInput").ap()
    maskT = nc.dram_tensor("maskT", [P, 4, 512], BF16, kind="ExternalInput").ap()
    out = nc.dram_tensor("out", [NQ, D], F32, kind="ExternalOutput").ap()
    # cores 2b (h=0) and 2b+1 (h=1) of batch b exchange K/V halves
    GROUPS = [[0, 1], [2, 3], [4, 5], [6, 7]]

    with tile.TileContext(nc) as tc, ExitStack() as ctx:
        const = ctx.enter_context(tc.tile_pool(name="const", bufs=1))
        persist = ctx.enter_context(tc.tile_pool(name="persist", bufs=1))

        ident = const.tile([P, P], F32)
        make_identity(nc, ident[:])
        ones = const.tile([P, 1], BF16)
        nc.gpsimd.memset(ones[:], 1.0)
        mask_sb = const.tile([P, 4, 512], BF16)

        QT = persist.tile([P, 8, NQ], BF16)    # [e_in_tile, e_tile, q]
        KT = persist.tile([P, 8, S], BF16)     # [e_in_tile, e_tile, k]
        V = persist.tile([P, 16, D], BF16)     # [k_in_tile, k_tile, e]
        denT = persist.tile([P, 2 * NSLOT], F32)
        rinv = persist.tile([P, 2 * NSLOT], F32)

        # ---------------- projection phases ----------------
        # K (and Q) are projected in full on every core — keeping scores
        # off the collective critical path — while V is projected only for
        # the core's own four 256-row query blocks {2t+h} (so the xq tiles
        # double as the V stationary operand) and pair-wise all-gathered in
        # two stages.  Gather output is rank-ordered: stage s, rank r,
        # piece j holds global block 4s+2j+r on both cores — SPMD-uniform.
        # V-own runs first so the collectives launch early; their latency
        # hides under the K/Q projections.
        with tc.tile_pool(name="wsb", bufs=1) as wsb_pool, \
             tc.tile_pool(name="stage", bufs=2) as stage_pool, \
             tc.tile_pool(name="xc", bufs=6) as xc_pool, \
             tc.tile_pool(name="kvh", bufs=1) as kvh_pool, \
             tc.tile_pool(name="ccdram", bufs=2, space="DRAM") as ccdram, \
             tc.tile_pool(name="pproj", bufs=4, space="PSUM") as pproj:

            def load_cast(dst, dram_chunk):
                st = stage_pool.tile([P, 8, 256], F32, tag="stage")
                nc.sync.dma_start(st[:], dram_chunk)
                nc.vector.tensor_copy(dst, st[:])

            wq_sb = wsb_pool.tile([P, 8, D], BF16, tag="wq")
            wk_sb = wsb_pool.tile([P, 8, D], BF16, tag="wk")
            wv_sb = wsb_pool.tile([P, 8, D], BF16, tag="wv")

            def load_chunk_pair(dram4, i):
                xc = xc_pool.tile([P, 8, 512], BF16, tag="xc")
                for s2 in range(2):
                    load_cast(xc[:, :, 256 * s2:256 * (s2 + 1)],
                              dram4[2 * i + s2])
                return xc

            # Loads are emitted lazily, right before the compute group that
            # consumes them, so the DVE cast/evac FIFO order matches true
            # data-arrival order (no head-of-line blocking).

            def emit_v(s, xc):
                # V for the core's own query blocks of stage s (its blocks
                # 4s+h and 4s+2+h, i.e. xq chunk s) + pair all-gather
                vh = kvh_pool.tile([P, 4, D], BF16, tag="vh")
                for ec in range(2):
                    for k4 in range(4):
                        ps = pproj.tile([P, 512], F32, tag="pp")
                        for dt in range(8):
                            nc.tensor.matmul(
                                ps[:], xc[:, dt, P * k4:P * (k4 + 1)],
                                wv_sb[:, dt, 512 * ec:512 * (ec + 1)],
                                start=(dt == 0), stop=(dt == 7))
                        nc.scalar.copy(vh[:, k4, 512 * ec:512 * (ec + 1)],
                                       ps[:])
                cc_in = ccdram.tile([P, 4096], BF16, tag="cc_in")
                cc_out = ccdram.tile([2, P, 4096], BF16, tag="cc_out")
                # all collective-adjacent DMAs live on the gpsimd queue so
                # they never head-of-line-block the input-load sync queue
                nc.gpsimd.dma_start(cc_in[:],
                                    vh[:].rearrange("p a b -> p (a b)"))
                nc.gpsimd.collective_compute(
                    "AllGather", mybir.AluOpType.bypass,
                    replica_groups=GROUPS,
                    ins=[cc_in[:]], outs=[cc_out[:]])
                # stage s, rank r, piece j -> global 256-row block 4s+2j+r
                for r in range(2):
                    co = cc_out[r].rearrange("p (a b) -> p a b", a=4)
                    for j in range(2):
                        g = 4 * s + 2 * j + r
                        nc.gpsimd.dma_start(V[:, 2 * g:2 * g + 2, :],
                                            co[:, 2 * j:2 * j + 2, :])

            def emit_k(kc, xc):
                for et in range(8):
                    ps = pproj.tile([P, 512], F32, tag="pp")
                    for dt in range(8):
                        nc.tensor.matmul(
                            ps[:], wk_sb[:, dt, P * et:P * (et + 1)],
                            xc[:, dt, :],
                            start=(dt == 0), stop=(dt == 7))
                    nc.scalar.copy(KT[:, et, 512 * kc:512 * (kc + 1)], ps[:])

            def emit_q(qc, xqc):
                for et in range(8):
                    ps = pproj.tile([P, 512], F32, tag="pp")
                    for dt in range(8):
                        nc.tensor.matmul(
                            ps[:], wq_sb[:, dt, P * et:P * (et + 1)],
                            xqc[:, dt, :],
                            start=(dt == 0), stop=(dt == 7))
                    nc.scalar.copy(QT[:, et, 512 * qc:512 * (qc + 1)], ps[:])

            # eager loads, in consumption order; PE order matches arrival
            for c in range(2):
                load_cast(wv_sb[:, :, 256 * c:256 * (c + 1)], wv4[c])
            xq_tiles = [load_chunk_pair(xq4, 0)]
            for c in range(2, 4):
                load_cast(wv_sb[:, :, 256 * c:256 * (c + 1)], wv4[c])
            xq_tiles.append(load_chunk_pair(xq4, 1))
            for c in range(4):
                load_cast(wq_sb[:, :, 256 * c:256 * (c + 1)], wq4[c])
            for c in range(4):
                load_cast(wk_sb[:, :, 256 * c:256 * (c + 1)], wk4[c])
            xg_tiles = [load_chunk_pair(xt4, 0)]

            nc.sync.dma_start(mask_sb[:], maskT[:])
            emit_v(0, xq_tiles[0])
            emit_v(1, xq_tiles[1])
            emit_q(0, xq_tiles[0])
            emit_q(1, xq_tiles[1])
            emit_k(0, xg_tiles[0])
            xg_tiles.append(load_chunk_pair(xt4, 1))
            emit_k(1, xg_tiles[1])
            xg_tiles.append(load_chunk_pair(xt4, 2))
            emit_k(2, xg_tiles[2])
            xg_tiles.append(load_chunk_pair(xt4, 3))
            emit_k(3, xg_tiles[3])

        # ---------------- attention phase ----------------
        # Slots are processed in pairs (2p, 2p+1).  The pair shares k-tiles
        # 0..nsh-1 (nsh = 4(2p+1), slot 2p's full extent), so scores / exp /
        # PV over that range run at N=512 covering both slots' queries;
        # slot 2p+1's four extra k-tiles run at N=256 on the right half.
        # PV is deferred to per-pair sweeps over the retained exp tiles in
        # four e-tile-pair passes, so every psum accumulation region owns a
        # full bank.
        with tc.tile_pool(name="ps_s", bufs=3, space="PSUM") as ps_s, \
             tc.tile_pool(name="ps_d", bufs=1, space="PSUM") as ps_d, \
             tc.tile_pool(name="ps_o", bufs=1, space="PSUM") as ps_o, \
             tc.tile_pool(name="ps_t", bufs=2, space="PSUM") as ps_t, \
             tc.tile_pool(name="expp", bufs=2) as expp, \
             tc.tile_pool(name="tmpp", bufs=2) as tmpp, \
             tc.tile_pool(name="obuf", bufs=5) as obufp, \
             tc.tile_pool(name="osb", bufs=5) as osbp, \
             tc.tile_pool(name="dendram", bufs=1, space="DRAM") as dendramp, \
             tc.tile_pool(name="dsb", bufs=2) as dsbp:

            den_dram = dendramp.tile([2, 2 * QB], F32)
            post_queue = deque()

            def pop_post():
                if post_queue:
                    post_queue.popleft()()

            for p in range(2):
                nsh = 4 * (2 * p + 1)      # shared k-tiles (slot 2p extent)
                ntot = nsh + 4             # + slot 2p+1's extra k-tiles
                expbuf = expp.tile([P, 16, 512], BF16, tag="expbuf")
                pd = ps_d.tile([P, 512], F32, tag="pd")

                # denominator matmul for tile k; emitted one iteration late
                # (software pipeline) so PE never waits on the exp latency
                def emit_den(k, pd=pd, nsh=nsh, ntot=ntot, expbuf=expbuf):
                    if k < nsh:
                        nc.tensor.matmul(pd[0:1, :], ones[:, 0:1],
                                         expbuf[:, k, :],
                                         start=(k == 0), stop=False)
                    else:
                        nc.tensor.matmul(pd[0:1, 256:512], ones[:, 0:1],
                                         expbuf[:, k, 0:256],
                                         start=False, stop=(k == ntot - 1))

                # scores + exp over the shared range at N=512
                for kt in range(nsh):
                    ps = ps_s.tile([P, 512], F32, tag="ps")
                    for et in range(8):
                        nc.tensor.matmul(
                            ps[:], KT[:, et, P * kt:P * (kt + 1)],
                            QT[:, et, 512 * p:512 * (p + 1)],
                            start=(et == 0), stop=(et == 7))
                    j = kt - (nsh - 4)
                    if j >= 0:
                        tmp = tmpp.tile([P, 512], BF16, tag="tmp")
                        nc.scalar.activation(tmp[:], ps[:],
                                             mybir.ActivationFunctionType.Exp,
                                             scale=SCALE)
                        nc.vector.tensor_tensor(expbuf[:, kt, :], tmp[:],
                                                mask_sb[:, j, :],
                                                mybir.AluOpType.mult)
                    else:
                        nc.scalar.activation(expbuf[:, kt, :], ps[:],
                                             mybir.ActivationFunctionType.Exp,
                                             scale=SCALE)
                    if kt >= 1:
                        emit_den(kt - 1)
                    pop_post()
                    pop_post()

                # slot 2p+1's extra k-tiles at N=256 (right half)
                for ex in range(4):
                    kt = nsh + ex
                    ps = ps_s.tile([P, 512], F32, tag="ps")
                    for et in range(8):
                        nc.tensor.matmul(
                            ps[:, 0:256], KT[:, et, P * kt:P * (kt + 1)],
                            QT[:, et, 512 * p + 256:512 * (p + 1)],
                            start=(et == 0), stop=(et == 7))
                    tmp = tmpp.tile([P, 512], BF16, tag="tmp")
                    nc.scalar.activation(tmp[:, 0:256], ps[:, 0:256],
                                         mybir.ActivationFunctionType.Exp,
                                         scale=SCALE)
                    nc.vector.tensor_tensor(expbuf[:, kt, 0:256],
                                            tmp[:, 0:256],
                                            mask_sb[:, ex, 0:256],
                                            mybir.AluOpType.mult)
                    emit_den(kt - 1)
                    pop_post()
                    pop_post()
                emit_den(ntot - 1)

                # denominator -> [q,1] layout via DRAM roundtrip
                dsb = dsbp.tile([1, 512], F32, tag="den")
                nc.vector.tensor_copy(dsb[:], pd[0:1, :])
                nc.sync.dma_start(den_dram[p:p + 1, :], dsb[:])
                nc.sync.dma_start(
                    denT[:, 4 * p:4 * p + 4],
                    den_dram[p:p + 1, :].rearrange("o (c q) -> (o q) c", q=P))
                nc.vector.reciprocal(rinv[:, 4 * p:4 * p + 4],
                                     denT[:, 4 * p:4 * p + 4])

                # PV in four e-tile-pair passes over the retained exp tiles
                obufs = []
                for a in range(4):
                    po = ps_o.tile([P, 2, 512], F32, tag="po")
                    for kt in range(nsh):
                        for i in range(2):
                            et = 2 * a + i
                            nc.tensor.matmul(
                                po[:, i, :], V[:, kt, P * et:P * (et + 1)],
                                expbuf[:, kt, :],
                                start=(kt == 0), stop=False)
                    for ex in range(4):
                        kt = nsh + ex
                        for i in range(2):
                            et = 2 * a + i
                            nc.tensor.matmul(
                                po[:, i, 256:512],
                                V[:, kt, P * et:P * (et + 1)],
                                expbuf[:, kt, 0:256],
                                start=False, stop=(ex == 3))
                    ob = obufp.tile([P, 2, 512], F32, tag="ob")
                    nc.vector.tensor_copy(ob[:], po[:])
                    obufs.append(ob)
                    pop_post()
                    pop_post()

                # queue transpose + normalize + output-store work.
                # a-major order so each obuf's readers run consecutively
                # (its release never hides behind a blocked later pop).
                osbs = [osbp.tile([P, D], F32, tag="osb", name=f"osb{p}{qh}")
                        for qh in range(4)]
                for a in range(4):
                    for qh in range(4):
                        for i in range(2):
                            def mk_tr(a=a, i=i, qh=qh, osb=osbs[qh], p=p,
                                      ob=obufs[a]):
                                def doit():
                                    tr = ps_t.tile([P, P], F32, tag="tr")
                                    nc.tensor.transpose(
                                        tr[:], ob[:, i, P * qh:P * (qh + 1)],
                                        ident[:])
                                    nc.scalar.mul(
                                        osb[:, P * (2 * a + i):
                                            P * (2 * a + i + 1)], tr[:],
                                        rinv[:, 4 * p + qh:4 * p + qh + 1])
                                return doit
                            post_queue.append(mk_tr())
                for qh in range(4):
                    def mk_out(osb=osbs[qh], p=p, qh=qh):
                        def doit():
                            r0 = P * (4 * p + qh)
                            nc.sync.dma_start(out[r0:r0 + P, :], osb[:])
                        return doit
                    post_queue.append(mk_out())

            while post_queue:
                post_queue.popleft()()

    nc.compile()
    return nc


_NC_CACHE = None


def _get_nc():
    global _NC_CACHE
    if _NC_CACHE is None:
        _NC_CACHE = _build_kernel()
    return _NC_CACHE


def _make_masks():
    kk = np.arange(P)[:, None]
    qq = np.arange(256)[None, :]
    diag0 = (qq >= kk).astype(np.float32)
    diag1 = (qq >= kk + P).astype(np.float32)
    m = {}
    for h in range(2):
        mt = np.zeros((P, 4, 512), np.float32)
        mt[:, :, 256:] = 1.0  # right half (the later slot of a pair): allowed
        if h == 0:
            mt[:, 0, :256], mt[:, 1, :256] = diag0, diag1
        else:
            mt[:, 0, :256], mt[:, 1, :256] = 1.0, 1.0
            mt[:, 2, :256], mt[:, 3, :256] = diag0, diag1
        m[h] = mt.astype(ml_dtypes.bfloat16)
    return m


def _prep_inputs(x, Wq, Wk, Wv):
    def w4(W):
        return np.ascontiguousarray(
            W.reshape(8, P, 4, 256).transpose(2, 1, 0, 3))

    wq4, wk4, wv4 = w4(Wq), w4(Wk), w4(Wv)
    masks = _make_masks()
    in_maps = []
    for core in range(NCORES):
        b, h = divmod(core, 2)
        xb = np.asarray(x[b])
        xt4 = np.ascontiguousarray(
            xb.reshape(8, 256, 8, P).transpose(0, 3, 2, 1))
        order = np.concatenate(
            [np.arange(QB * (2 * t + h), QB * (2 * t + h) + QB)
             for t in range(NSLOT)])
        xq = xb[order]
        xq4 = np.ascontiguousarray(
            xq.reshape(4, 256, 8, P).transpose(0, 3, 2, 1))
        in_maps.append({
            "xt4": xt4, "xq4": xq4,
            "wq4": wq4, "wk4": wk4, "wv4": wv4,
            "maskT": masks[h],
        })
    return in_maps


def run(inputs, trace=False):
    nc = _get_nc()
    in_maps = _prep_inputs(inputs["x"], inputs["Wq"], inputs["Wk"],
                           inputs["Wv"])
    res = bass_utils.run_bass_kernel_spmd(
        nc, in_maps, core_ids=list(range(NCORES)), trace=trace)
    out = np.empty((B, S, D), np.float32)
    for core in range(NCORES):
        b, h = divmod(core, 2)
        oc = res.results[core]["out"]
        for t in range(NSLOT):
            out[b, QB * (2 * t + h):QB * (2 * t + h) + QB] = \
                oc[QB * t:QB * t + QB]
    return out, res


def kernel(**inputs):
    out, _ = run(inputs, trace=False)
    return out



# revision 2
# speedup vs baseline: 1.2653x; 1.2653x over previous
"""Causal single-head attention (B=4, S=2048, D=1024) on 8 trn2 NeuronCores.

Sharding: core = (batch b, parity h).  Each core owns the 1024 queries of
batch b in 256-row blocks {2t+h : t=0..3} (interleaved for causal load
balance), projects Q for its own rows, K for the full sequence, and V for
its own rows only (V halves are pair-wise all-gathered, hidden under the
K projection).

All inputs are pre-cast to bf16 on the host (the on-chip matmuls are bf16
anyway), halving input HBM traffic and removing the f32 staging casts.

On-chip dataflow (per core, SPMD-uniform):
  proj:  V_own[k,e] = xq^T Wv   (8 row-tiles; gathered pair-wise via one
                                 AllGather into V4[r][j])
         QT[e,q]    = Wq^T xq^T
         KT[e,k]    = Wk^T x^T   (full sequence, 4 key chunks of 512)
  attention per slot pair p (queries 512p..512p+511, nsh = 4(2p+1)):
         scoresT[k,q] = KT^T QT  over k-tiles 0..nsh-1 at N=512
                        (+ 4 extra k-tiles at N=256 for the later slot)
         expT = exp(scoresT/32) * mask    (multiplicative 0/1 causal mask)
         den[1,q] += ones^T expT          (matmul; [q]-major via DRAM
                                           roundtrip -> rinv[q,1])
         out[q,e]  = sum_k expT[k,q]^T V[k,e]   <- PV with expT stationary,
                     accumulated per 128-query tile directly in [q,e]
                     orientation (no PE transposes), scaled by rinv on DVE
                     and stored bf16.
"""

import os
import sys
from contextlib import ExitStack

import numpy as np
import ml_dtypes

import concourse.bass as bass
import concourse.mybir as mybir
import concourse.tile as tile
from concourse import bacc
from concourse import bass_utils

B, S, D = 4, 2048, 1024
P = 128
QB = 256          # queries per slot
NSLOT = 4         # slots per core
NQ = QB * NSLOT   # queries per core
NCORES = 8
F32 = mybir.dt.float32
BF16 = mybir.dt.bfloat16
SCALE = 1.0 / 32.0  # 1/sqrt(D)


def _build_kernel():
    nc = bacc.Bacc("TRN2", target_bir_lowering=False, debug=False,
                   num_devices=NCORES)

    xtd = nc.dram_tensor("xtd", [4, P, 8, 512], BF16, kind="ExternalInput").ap()
    xqd = nc.dram_tensor("xqd", [2, P, 8, 512], BF16, kind="ExternalInput").ap()
    wqd = nc.dram_tensor("wqd", [P, 8, D], BF16, kind="ExternalInput").ap()
    wkd = nc.dram_tensor("wkd", [P, 8, D], BF16, kind="ExternalInput").ap()
    wvd = nc.dram_tensor("wvd", [2, P, 8, 512], BF16, kind="ExternalInput").ap()
    maskT = nc.dram_tensor("maskT", [P, 4, 512], BF16, kind="ExternalInput").ap()
    out = nc.dram_tensor("out", [NQ, D], BF16, kind="ExternalOutput").ap()
    # cores 2b (h=0) and 2b+1 (h=1) of batch b exchange V halves
    GROUPS = [[0, 1], [2, 3], [4, 5], [6, 7]]

    with tile.TileContext(nc) as tc, ExitStack() as ctx:
        const = ctx.enter_context(tc.tile_pool(name="const", bufs=1))
        persist = ctx.enter_context(tc.tile_pool(name="persist", bufs=1))

        ones = const.tile([P, 1], BF16)
        nc.gpsimd.memset(ones[:], 1.0)
        mask_sb = const.tile([P, 4, 512], BF16)

        QT = persist.tile([P, 8, NQ], BF16)      # [e_in_tile, e_tile, q]
        KT = persist.tile([P, 8, S], BF16)       # [e_in_tile, e_tile, k]
        V4 = persist.tile([P, 2, 8, D], BF16)    # [k_in_tile, parity, j, e]
        denT = persist.tile([P, 2 * NSLOT], F32)
        rinv = persist.tile([P, 2 * NSLOT], F32)

        # ---------------- projection phase ----------------
        # V-own first so the pair AllGather launches as early as possible;
        # its latency hides under the Q and K projections.  K is projected
        # in full on every core (a K-gather would put ~50us of collective
        # latency on the critical path; 27us of extra PE work is cheaper).
        with tc.tile_pool(name="wsb", bufs=1) as wsb_pool, \
             tc.tile_pool(name="xtp", bufs=2) as xt_pool, \
             tc.tile_pool(name="xqp", bufs=1) as xq_pool, \
             tc.tile_pool(name="vhp", bufs=1) as vh_pool, \
             tc.tile_pool(name="ccdram", bufs=1, space="DRAM") as ccdram, \
             tc.tile_pool(name="pproj", bufs=4, space="PSUM") as pproj:

            wq_sb = wsb_pool.tile([P, 8, D], BF16, tag="wq")
            wk_sb = wsb_pool.tile([P, 8, D], BF16, tag="wk")
            wv_sb = wsb_pool.tile([P, 2, 8, 512], BF16, tag="wv")
            xq_sb = xq_pool.tile([P, 2, 8, 512], BF16, tag="xq")
            vh = vh_pool.tile([P, 8, D], BF16, tag="vh")

            # input loads, issued in consumption order (sync/HWDGE queue)
            nc.sync.dma_start(wv_sb[:, 0], wvd[0])
            nc.sync.dma_start(xq_sb[:, 0], xqd[0])
            nc.sync.dma_start(xq_sb[:, 1], xqd[1])
            nc.sync.dma_start(wv_sb[:, 1], wvd[1])
            nc.sync.dma_start(wq_sb[:], wqd[:])
            nc.sync.dma_start(wk_sb[:], wkd[:])
            xt_tiles = []
            for c in range(4):
                xt = xt_pool.tile([P, 8, 512], BF16, tag="xt")
                nc.sync.dma_start(xt[:], xtd[c])
                xt_tiles.append(xt)
            nc.sync.dma_start(mask_sb[:], maskT[:])

            # V_own[k,e]: stationary xq row-tile, moving wv e-half
            for eh in range(2):
                for hh in range(2):
                    for jj in range(4):
                        j = 4 * hh + jj
                        ps = pproj.tile([P, 512], F32, tag="pp")
                        for dt in range(8):
                            nc.tensor.matmul(
                                ps[:],
                                xq_sb[:, hh, dt, P * jj:P * (jj + 1)],
                                wv_sb[:, eh, dt, :],
                                start=(dt == 0), stop=(dt == 7))
                        nc.scalar.copy(vh[:, j, 512 * eh:512 * (eh + 1)],
                                       ps[:])

            # pair all-gather of V halves (runs on TOPSP/SDMA, overlapped)
            cc_in = ccdram.tile([P, 8 * D], BF16, tag="cc_in")
            cc_out = ccdram.tile([2, P, 8 * D], BF16, tag="cc_out")
            nc.gpsimd.dma_start(cc_in[:],
                                vh[:].rearrange("p a b -> p (a b)"))
            nc.gpsimd.collective_compute(
                "AllGather", mybir.AluOpType.bypass,
                replica_groups=GROUPS,
                ins=[cc_in[:]], outs=[cc_out[:]])
            for r in range(2):
                nc.gpsimd.dma_start(
                    V4[:, r].rearrange("p a b -> p (a b)"), cc_out[r])

            # QT[e,q]: stationary wq e-tile, moving xq half
            for et in range(8):
                for qh in range(2):
                    ps = pproj.tile([P, 512], F32, tag="pp")
                    for dt in range(8):
                        nc.tensor.matmul(
                            ps[:], wq_sb[:, dt, P * et:P * (et + 1)],
                            xq_sb[:, qh, dt, :],
                            start=(dt == 0), stop=(dt == 7))
                    nc.scalar.copy(QT[:, et, 512 * qh:512 * (qh + 1)], ps[:])

            # KT[e,k]: full sequence, chunk by chunk as loads arrive
            for c in range(4):
                for et in range(8):
                    ps = pproj.tile([P, 512], F32, tag="pp")
                    for dt in range(8):
                        nc.tensor.matmul(
                            ps[:], wk_sb[:, dt, P * et:P * (et + 1)],
                            xt_tiles[c][:, dt, :],
                            start=(dt == 0), stop=(dt == 7))
                    nc.scalar.copy(KT[:, et, 512 * c:512 * (c + 1)], ps[:])

        # ---------------- attention phase ----------------
        # Slot pairs (2p, 2p+1) share k-tiles 0..nsh-1 at N=512; the later
        # slot's 4 extra k-tiles run at N=256 on the right half.  PV runs
        # with expT tiles stationary, producing out[q,e] directly (no PE
        # transposes); the softmax denominator is folded in via a DVE
        # tensor_scalar multiply during psum evacuation.
        with tc.tile_pool(name="ps_s", bufs=3, space="PSUM") as ps_s, \
             tc.tile_pool(name="ps_d", bufs=1, space="PSUM") as ps_d, \
             tc.tile_pool(name="ps_o", bufs=2, space="PSUM") as ps_o, \
             tc.tile_pool(name="expp", bufs=2) as expp, \
             tc.tile_pool(name="tmpp", bufs=2) as tmpp, \
             tc.tile_pool(name="osb", bufs=4) as osbp, \
             tc.tile_pool(name="dendram", bufs=1, space="DRAM") as dendramp, \
             tc.tile_pool(name="dsb", bufs=2) as dsbp:

            den_dram = dendramp.tile([2, 2 * QB], F32)

            for p in range(2):
                nsh = 4 * (2 * p + 1)      # shared k-tiles (slot 2p extent)
                ntot = nsh + 4             # + slot 2p+1's extra k-tiles
                expbuf = expp.tile([P, 16, 512], BF16, tag="expbuf")
                pd = ps_d.tile([P, 512], F32, tag="pd")

                # denominator matmul for tile k; emitted one iteration late
                # (software pipeline) so PE never waits on the exp latency
                def emit_den(k, pd=pd, nsh=nsh, ntot=ntot, expbuf=expbuf):
                    if k < nsh:
                        nc.tensor.matmul(pd[0:1, :], ones[:, 0:1],
                                         expbuf[:, k, :],
                                         start=(k == 0), stop=False)
                    else:
                        nc.tensor.matmul(pd[0:1, 256:512], ones[:, 0:1],
                                         expbuf[:, k, 0:256],
                                         start=False, stop=(k == ntot - 1))

                # scores + exp over the shared range at N=512
                for kt in range(nsh):
                    ps = ps_s.tile([P, 512], F32, tag="ps")
                    for et in range(8):
                        nc.tensor.matmul(
                            ps[:], KT[:, et, P * kt:P * (kt + 1)],
                            QT[:, et, 512 * p:512 * (p + 1)],
                            start=(et == 0), stop=(et == 7))
                    j = kt - (nsh - 4)
                    if j >= 0:
                        tmp = tmpp.tile([P, 512], BF16, tag="tmp")
                        nc.scalar.activation(tmp[:], ps[:],
                                             mybir.ActivationFunctionType.Exp,
                                             scale=SCALE)
                        nc.vector.tensor_tensor(expbuf[:, kt, :], tmp[:],
                                                mask_sb[:, j, :],
                                                mybir.AluOpType.mult)
                    else:
                        nc.scalar.activation(expbuf[:, kt, :], ps[:],
                                             mybir.ActivationFunctionType.Exp,
                                             scale=SCALE)
                    if kt >= 1:
                        emit_den(kt - 1)

                # slot 2p+1's extra k-tiles at N=256 (right half)
                for ex in range(4):
                    kt = nsh + ex
                    ps = ps_s.tile([P, 512], F32, tag="ps")
                    for et in range(8):
                        nc.tensor.matmul(
                            ps[:, 0:256], KT[:, et, P * kt:P * (kt + 1)],
                            QT[:, et, 512 * p + 256:512 * (p + 1)],
                            start=(et == 0), stop=(et == 7))
                    tmp = tmpp.tile([P, 512], BF16, tag="tmp")
                    nc.scalar.activation(tmp[:, 0:256], ps[:, 0:256],
                                         mybir.ActivationFunctionType.Exp,
                                         scale=SCALE)
                    nc.vector.tensor_tensor(expbuf[:, kt, 0:256],
                                            tmp[:, 0:256],
                                            mask_sb[:, ex, 0:256],
                                            mybir.AluOpType.mult)
                    emit_den(kt - 1)
                emit_den(ntot - 1)

                # denominator -> [q,1] layout via DRAM roundtrip
                dsb = dsbp.tile([1, 512], F32, tag="den")
                nc.vector.tensor_copy(dsb[:], pd[0:1, :])
                nc.sync.dma_start(den_dram[p:p + 1, :], dsb[:])
                nc.sync.dma_start(
                    denT[:, 4 * p:4 * p + 4],
                    den_dram[p:p + 1, :].rearrange("o (c q) -> (o q) c", q=P))
                nc.vector.reciprocal(rinv[:, 4 * p:4 * p + 4],
                                     denT[:, 4 * p:4 * p + 4])

                # PV per 128-query tile: expT stationary, V moving; psum
                # accumulates out[q, 1024e] across the tile's k range
                for qt in range(4):
                    kcnt = nsh if qt < 2 else ntot
                    po = ps_o.tile([P, 2, 512], F32, tag="po")
                    for kt in range(kcnt):
                        qo = P * qt if kt < nsh else P * (qt - 2)
                        t, rem = divmod(kt, 4)
                        r, jj = divmod(rem, 2)
                        jv = 2 * t + jj
                        for eh in range(2):
                            nc.tensor.matmul(
                                po[:, eh, :],
                                expbuf[:, kt, qo:qo + P],
                                V4[:, r, jv, 512 * eh:512 * (eh + 1)],
                                start=(kt == 0), stop=(kt == kcnt - 1))
                    ob = osbp.tile([P, D], BF16, tag="ob")
                    nc.vector.tensor_scalar_mul(
                        ob[:], po[:].rearrange("p a b -> p (a b)"),
                        rinv[:, 4 * p + qt:4 * p + qt + 1])
                    r0 = 512 * p + P * qt
                    nc.sync.dma_start(out[r0:r0 + P, :], ob[:])

    nc.compile()
    return nc


_NC_CACHE = None


def _get_nc():
    global _NC_CACHE
    if _NC_CACHE is None:
        _NC_CACHE = _build_kernel()
    return _NC_CACHE


def _make_masks():
    kk = np.arange(P)[:, None]
    qq = np.arange(256)[None, :]
    diag0 = (qq >= kk).astype(np.float32)
    diag1 = (qq >= kk + P).astype(np.float32)
    m = {}
    for h in range(2):
        mt = np.zeros((P, 4, 512), np.float32)
        mt[:, :, 256:] = 1.0  # right half (the later slot of a pair): allowed
        if h == 0:
            mt[:, 0, :256], mt[:, 1, :256] = diag0, diag1
        else:
            mt[:, 0, :256], mt[:, 1, :256] = 1.0, 1.0
            mt[:, 2, :256], mt[:, 3, :256] = diag0, diag1
        m[h] = mt.astype(ml_dtypes.bfloat16)
    return m


def _prep_inputs(x, Wq, Wk, Wv):
    bf16 = ml_dtypes.bfloat16

    def wfull(W):
        # [d_in, e] -> [p, dt, e]
        return np.ascontiguousarray(
            np.asarray(W, np.float32).reshape(8, P, D).transpose(1, 0, 2)
        ).astype(bf16)

    def whalves(W):
        # [d_in, e] -> [eh, p, dt, 512]
        wf = np.asarray(W, np.float32).reshape(8, P, 2, 512)
        return np.ascontiguousarray(wf.transpose(2, 1, 0, 3)).astype(bf16)

    wq4, wk4, wv4 = wfull(Wq), wfull(Wk), whalves(Wv)
    masks = _make_masks()
    in_maps = []
    for core in range(NCORES):
        b, h = divmod(core, 2)
        xb = np.asarray(x[b], np.float32)
        xt4 = np.ascontiguousarray(
            xb.reshape(4, 512, 8, P).transpose(0, 3, 2, 1)).astype(bf16)
        order = np.concatenate(
            [np.arange(QB * (2 * t + h), QB * (2 * t + h) + QB)
             for t in range(NSLOT)])
        xq = xb[order]
        xq4 = np.ascontiguousarray(
            xq.reshape(2, 512, 8, P).transpose(0, 3, 2, 1)).astype(bf16)
        in_maps.append({
            "xtd": xt4, "xqd": xq4,
            "wqd": wq4, "wkd": wk4, "wvd": wv4,
            "maskT": masks[h],
        })
    return in_maps


def run(inputs, trace=False):
    nc = _get_nc()
    in_maps = _prep_inputs(inputs["x"], inputs["Wq"], inputs["Wk"],
                           inputs["Wv"])
    res = bass_utils.run_bass_kernel_spmd(
        nc, in_maps, core_ids=list(range(NCORES)), trace=trace)
    out = np.empty((B, S, D), np.float32)
    for core in range(NCORES):
        b, h = divmod(core, 2)
        oc = np.asarray(res.results[core]["out"]).astype(np.float32)
        for t in range(NSLOT):
            out[b, QB * (2 * t + h):QB * (2 * t + h) + QB] = \
                oc[QB * t:QB * t + QB]
    return out, res


def kernel(**inputs):
    out, _ = run(inputs, trace=False)
    return out
